# revision 9
# baseline (speedup 1.0000x reference)
"""Causal self-attention with KV cache on 8 Trainium2 NeuronCores.

Strategy: tensor-parallel over heads (16 heads / 8 cores = 2 heads per core).
  Per core:
    1. qkv^T projection: q^T,k^T per head (cols-on-partitions) and v (natural
       layout) via fp32r (TF32) matmuls against host-pre-transposed x^T.
    2. Flash-style attention in S^T layout (keys on partitions): S^T = k^T.T@q^T,
       exp on ACT (scale folded), causal mask via 0/1 mask multiply on diagonal
       blocks only, O^T = v.T@expS and row-sums l via ones-matmul, both
       accumulated in PSUM; normalize with reciprocal(l).
    3. AllToAll (2 MB/core) reshards O from head-parallel to sequence-parallel.
    4. Output projection with the full W_proj on each core for its 256 tokens,
       bias folded into the PSUM->SBUF eviction.
  Host: shards/pre-transposes inputs, assembles/transposes outputs, passes the
  KV cache through unchanged.
"""
import numpy as np

import concourse.bass as bass
import concourse.tile as tile
from concourse import bacc, mybir
from concourse.bass_utils import run_bass_kernel_spmd

P = 128
NC = 8                     # cores
B, T, C = 2, 1024, 2048
H = 16
D = C // H                 # head dim 128
HL = H // NC               # heads per core = 2
Tc = 1024                  # cached tokens
KC = C // P                # contraction chunks = 16
TQ_TILE = 512
N_TQ = (B * T) // TQ_TILE  # 4 flat token tiles
F32R = mybir.dt.float32r
F32 = mybir.dt.float32

_BUILD_CACHE = {}


def _build(seq_start: int, debug: bool = False):
    """Build the single SPMD program (all cores run it with different data)."""
    assert seq_start >= Tc - 1, "kernel assumes the whole cache is visible"
    delta = seq_start - Tc  # extra causal offset for new keys (0 in practice)
    n_new_blocks = T // P   # 8 new-key blocks per (head,batch)

    def visible_blocks(tq2: int):
        """tk blocks (of 16 = 8 cache + 8 new) visible to query tile tq2."""
        q_max = tq2 * TQ_TILE + TQ_TILE - 1  # max query index in tile
        blocks = list(range(Tc // P))  # cache always fully visible
        for jn in range(n_new_blocks):
            if 128 * jn <= q_max + delta:  # at least one visible element
                blocks.append(Tc // P + jn)
        return blocks

    def is_diag(j: int, tq2: int):
        """block j needs masking against query tile tq2 (partially visible)."""
        if j < Tc // P:
            return False
        jn = j - Tc // P
        q_min = tq2 * TQ_TILE
        # fully visible iff max key (128jn+127) <= min query + delta
        return 128 * jn + 127 > q_min + delta

    nc = bacc.Bacc("TRN2", target_bir_lowering=False, debug=False, num_devices=NC)

    # ---- DRAM I/O (per-core shards; same program on every core) ----
    xt_d = nc.dram_tensor("xt", [P, KC, B * T], F32R, kind="ExternalInput").ap()
    wqk_d = nc.dram_tensor("wqk", [P, KC, 4 * P], F32R, kind="ExternalInput").ap()
    wv_d = nc.dram_tensor("wv", [P, KC, HL * P], F32R, kind="ExternalInput").ap()
    wp_d = nc.dram_tensor("wp", [KC, P, KC, P], F32R, kind="ExternalInput").ap()
    kct_d = nc.dram_tensor("kct", [HL, B, P, Tc], F32R, kind="ExternalInput").ap()
    vc_d = nc.dram_tensor("vc", [HL, B, Tc, D], F32R, kind="ExternalInput").ap()
    masks_d = nc.dram_tensor("masks", [P, 4, TQ_TILE], F32R, kind="ExternalInput").ap()
    ones_d = nc.dram_tensor("ones", [P, P], F32R, kind="ExternalInput").ap()
    bias_d = nc.dram_tensor("bias", [P, KC], F32, kind="ExternalInput").ap()

    out_d = nc.dram_tensor("out_t", [C, (B * T) // NC], F32, kind="ExternalOutput").ap()
    kout_d = nc.dram_tensor("k_new_t", [HL, B, D, T], F32, kind="ExternalOutput").ap()
    vout_d = nc.dram_tensor("v_new", [HL, B, T, D], F32, kind="ExternalOutput").ap()

    a2a_in_d = nc.dram_tensor("a2a_in", [NC, HL * D, 256], F32).ap()
    a2a_out_d = nc.dram_tensor("a2a_out", [NC, HL * D, 256], F32).ap()
    if debug:
        dbg_expS_d = nc.dram_tensor("dbg_expS", [P, 2 * TQ_TILE], F32, kind="ExternalOutput").ap()
        dbg_rl_d = nc.dram_tensor("dbg_rl", [P, TQ_TILE], F32, kind="ExternalOutput").ap()
        dbg_oT_d = nc.dram_tensor("dbg_oT", [2, P, TQ_TILE], F32, kind="ExternalOutput").ap()
        dbg_a2a_d = nc.dram_tensor("dbg_a2a", [NC, HL * D, 256], F32, kind="ExternalOutput").ap()
        dbg_ofull_d = nc.dram_tensor("dbg_ofull", [P, KC, 256], F32, kind="ExternalOutput").ap()

    with tile.TileContext(nc) as tc:
        with (
            # persistent across phases
            tc.tile_pool(name="persist", bufs=1) as persist,
            tc.tile_pool(name="wp_pool", bufs=4) as wp_pool,
            tc.tile_pool(name="cache_pool", bufs=2) as cache_pool,
            # phase 1 transients
            tc.tile_pool(name="xt_pool", bufs=6) as xt_pool,
            tc.tile_pool(name="w1_pool", bufs=4) as w1_pool,
            tc.tile_pool(name="evict_pool", bufs=3) as evict_pool,
            tc.tile_pool(name="ps_big", bufs=1, space="PSUM") as ps_big,
            tc.tile_pool(name="ps_sm", bufs=2, space="PSUM") as ps_sm,
            # phase 2
            tc.tile_pool(name="expS_pool", bufs=3) as expS_pool,
            tc.tile_pool(name="norm_pool", bufs=2) as norm_pool,
            # phase 3
            tc.tile_pool(name="out_pool", bufs=4) as out_pool,
        ):
            # ---------- resident tiles ----------
            qT = persist.tile([P, HL, B, T], F32R)          # q^T per (h,b)
            knew = persist.tile([P, HL, B, T], F32R)        # new k^T per (h,b)
            vnew = persist.tile([P, HL, B, T // P, D], F32R)  # new v blocks
            masks_t = persist.tile([P, 4, TQ_TILE], F32R)
            ones_t = persist.tile([P, P], F32R)
            bias_t = persist.tile([P, KC], F32)
            ofull = persist.tile([P, KC, 256], F32R)        # post-A2A O^T

            nc.sync.dma_start(masks_t[:], masks_d[:])
            nc.sync.dma_start(ones_t[:], ones_d[:])
            nc.sync.dma_start(bias_t[:], bias_d[:])

            # ---------- phase 1: qkv^T projection ----------
            for tqi in range(N_TQ):           # flat token tile: batch tqi//2
                b = tqi // 2
                q2 = tqi % 2
                # q/k accumulators packed in pairs: big tile i holds m=2i, 2i+1
                psum_big = [ps_big.tile([P, 2 * TQ_TILE], F32, name=f"big{i}", tag=f"big{i}")
                            for i in range(2)]
                psum_qk = [psum_big[m // 2][:, bass.ds((m % 2) * TQ_TILE, TQ_TILE)]
                           for m in range(4)]
                # one v token-chunk per PSUM bank (interleaved accumulation
                # groups within one bank lose writes on HW)
                psum_v = [ps_sm.tile([P, HL * P], F32, name=f"v{i}", tag=f"sm{i % 2}")
                          for i in range(4)]
                for kc in range(KC):
                    xt_t = xt_pool.tile([P, TQ_TILE], F32R, tag="xt")
                    nc.sync.dma_start(xt_t[:], xt_d[:, kc, bass.ds(tqi * TQ_TILE, TQ_TILE)])
                    wqk_t = w1_pool.tile([P, 4 * P], F32R, tag="wqk")
                    nc.sync.dma_start(wqk_t[:], wqk_d[:, kc])
                    wv_t = w1_pool.tile([P, HL * P], F32R, tag="wv")
                    nc.sync.dma_start(wv_t[:], wv_d[:, kc])
                    for m in range(4):        # q0 q1 k0 k1
                        nc.tensor.matmul(
                            psum_qk[m][:], wqk_t[:, bass.ts(m, P)], xt_t[:],
                            start=(kc == 0), stop=(kc == KC - 1),
                        )
                    for tcc in range(4):      # token chunks of 128
                        nc.tensor.matmul(
                            psum_v[tcc][:],
                            xt_t[:, bass.ts(tcc, P)], wv_t[:],
                            start=(kc == 0), stop=(kc == KC - 1),
                        )
                # evictions
                for h in range(HL):
                    nc.vector.tensor_copy(
                        qT[:, h, b, bass.ds(q2 * TQ_TILE, TQ_TILE)], psum_qk[h][:])
                    nc.vector.tensor_copy(
                        knew[:, h, b, bass.ds(q2 * TQ_TILE, TQ_TILE)], psum_qk[2 + h][:])
                    kout_t = evict_pool.tile([P, TQ_TILE], F32, tag="kout")
                    nc.vector.tensor_copy(kout_t[:], psum_qk[2 + h][:])
                    nc.sync.dma_start(
                        kout_d[h, b, :, bass.ds(q2 * TQ_TILE, TQ_TILE)], kout_t[:])
                for tcc in range(4):
                    blk = q2 * 4 + tcc        # new-token block index within batch
                    vps = psum_v[tcc][:]
                    for h in range(HL):
                        nc.vector.tensor_copy(
                            vnew[:, h, b, blk, :], vps[:, bass.ts(h, P)])
                    vout_t = evict_pool.tile([P, HL * P], F32, tag="vout")
                    nc.vector.tensor_copy(vout_t[:], vps[:])
                    for h in range(HL):
                        nc.sync.dma_start(
                            vout_d[h, b, bass.ds(blk * P, P), :], vout_t[:, bass.ts(h, P)])

            # ---------- phase 2: attention per (h, b) ----------
            scale = float(D) ** -0.5
            n_cache_blocks = Tc // P
            for h in range(HL):
                for b in range(B):
                    kc_t = cache_pool.tile([P, Tc], F32R, tag="kcache")
                    nc.sync.dma_start(kc_t[:], kct_d[h, b])
                    vc_t = cache_pool.tile([P, Tc // P, D], F32R, tag="vcache")
                    nc.sync.dma_start(
                        vc_t[:], vc_d[h, b].rearrange("(blk p) d -> p blk d", p=P))

                    def k_blk(j):
                        if j < n_cache_blocks:
                            return kc_t[:, bass.ts(j, P)]
                        return knew[:, h, b, bass.ts(j - n_cache_blocks, P)]

                    def v_blk(j):
                        if j < n_cache_blocks:
                            return vc_t[:, j, :]
                        return vnew[:, h, b, j - n_cache_blocks, :]

                    for tq2 in range(T // TQ_TILE):
                        vis = visible_blocks(tq2)
                        assert len(vis) % 2 == 0
                        pairs = [vis[i:i + 2] for i in range(0, len(vis), 2)]
                        q_rhs = qT[:, h, b, bass.ds(tq2 * TQ_TILE, TQ_TILE)]
                        psum_o = ps_sm.tile([P, TQ_TILE], F32, name="psum_o", tag="sm0")
                        psum_l = ps_sm.tile([P, TQ_TILE], F32, name="psum_l", tag="sm1")
                        first, last = vis[0], vis[-1]
                        for pi, pr in enumerate(pairs):
                            psum_s = ps_big.tile([P, 2 * TQ_TILE], F32,
                                                 name="psum_s", tag=f"big{pi % 2}")
                            for e, j in enumerate(pr):
                                nc.tensor.matmul(
                                    psum_s[:, bass.ts(e, TQ_TILE)], k_blk(j), q_rhs,
                                    start=True, stop=True)
                            expS = expS_pool.tile([P, 2 * TQ_TILE], F32R, tag="expS")
                            nc.scalar.activation(
                                expS[:], psum_s[:], mybir.ActivationFunctionType.Exp,
                                scale=scale)
                            if debug and h == 0 and b == 0 and tq2 == 0 and pi == 0:
                                nc.sync.dma_start(dbg_expS_d.bitcast(F32R)[:], expS[:])
                            for e, j in enumerate(pr):
                                if is_diag(j, tq2):
                                    m = (j - n_cache_blocks) - (tq2 * TQ_TILE) // P
                                    assert 0 <= m < 4
                                    nc.vector.tensor_mul(
                                        out=expS[:, bass.ts(e, TQ_TILE)],
                                        in0=expS[:, bass.ts(e, TQ_TILE)],
                                        in1=masks_t[:, m, :])
                            for e, j in enumerate(pr):
                                nc.tensor.matmul(
                                    psum_o[:], v_blk(j), expS[:, bass.ts(e, TQ_TILE)],
                                    start=(j == first), stop=(j == last))
                                nc.tensor.matmul(
                                    psum_l[:], ones_t[:], expS[:, bass.ts(e, TQ_TILE)],
                                    start=(j == first), stop=(j == last))
                        rl = norm_pool.tile([P, TQ_TILE], F32, tag="rl")
                        nc.vector.reciprocal(rl[:], psum_l[:])
                        oT = norm_pool.tile([P, TQ_TILE], F32R, tag="oT")
                        nc.vector.tensor_mul(out=oT[:], in0=psum_o[:], in1=rl[:])
                        if debug and h == 0 and b == 0 and tq2 == 0:
                            nc.sync.dma_start(dbg_rl_d[:], rl[:])
                        if debug and b == 0 and tq2 == 0:
                            nc.sync.dma_start(dbg_oT_d.bitcast(F32R)[h], oT[:])
                        # stage into A2A input: dest cores s0, s0+1
                        s0 = b * 4 + tq2 * 2
                        nc.sync.dma_start(
                            a2a_in_d.bitcast(F32R)[
                                bass.ds(s0, 2), bass.ds(h * D, D), :]
                            .rearrange("s p t -> p s t"),
                            oT.rearrange("p (s t) -> p s t", s=2))

            # ---------- A2A: head-parallel -> sequence-parallel ----------
            nc.gpsimd.collective_compute(
                "AllToAll", mybir.AluOpType.bypass,
                replica_groups=[list(range(NC))],
                ins=[a2a_in_d[:]], outs=[a2a_out_d[:]],
            )
            nc.sync.dma_start(
                ofull[:],
                a2a_out_d.bitcast(F32R).rearrange("s (o p) t -> p (s o) t", p=P))
            if debug:
                nc.gpsimd.dma_start(dbg_a2a_d[:], a2a_in_d[:])
                nc.sync.dma_start(dbg_ofull_d.bitcast(F32R)[:], ofull[:])

            # ---------- phase 3: output projection (my 256 tokens) ----------
            for col in range(KC):
                wp_t = wp_pool.tile([P, KC, P], F32R, tag="wp")
                nc.sync.dma_start(wp_t[:], wp_d[col])
                psum_c = ps_sm.tile([P, 256], F32, name="psum_c", tag=f"sm{col % 2}")
                for kc in range(KC):
                    nc.tensor.matmul(
                        psum_c[:], wp_t[:, kc, :], ofull[:, kc, :],
                        start=(kc == 0), stop=(kc == KC - 1))
                outsb = out_pool.tile([P, 256], F32, tag="osb")
                nc.scalar.activation(
                    outsb[:], psum_c[:], mybir.ActivationFunctionType.Identity,
                    bias=bias_t[:, bass.ds(col, 1)])
                nc.sync.dma_start(out_d[bass.ds(col * P, P), :], outsb[:])

    nc.finalize()
    return nc


def _get_program(seq_start: int, debug: bool = False):
    key = (seq_start, debug)
    if key not in _BUILD_CACHE:
        _BUILD_CACHE[key] = _build(seq_start, debug)
    return _BUILD_CACHE[key]


def _prep_inputs(x, cache_k, cache_v, W_qkv, W_proj, b_proj, seq_start):
    """Host-side sharding/layout prep. Returns in_maps for the 8 cores."""
    x = np.asarray(x, dtype=np.float32)
    cache_k = np.asarray(cache_k, dtype=np.float32)
    cache_v = np.asarray(cache_v, dtype=np.float32)
    W_qkv = np.asarray(W_qkv, dtype=np.float32)
    W_proj = np.asarray(W_proj, dtype=np.float32)
    b_proj = np.asarray(b_proj, dtype=np.float32)
    delta = int(seq_start) - Tc

    # x^T: (C, B*T) -> [P, KC, B*T]
    xt = np.ascontiguousarray(
        x.reshape(B * T, C).T.reshape(KC, P, B * T).transpose(1, 0, 2))
    # W_proj: [col, P(k-part), KC(k-chunk), P(col-elem)]
    wp = np.ascontiguousarray(
        W_proj.reshape(KC, P, KC, P).transpose(2, 1, 0, 3))
    # masks: [P, 4, TQ] 1.0 where key visible: c >= 128*m + r - delta
    r = np.arange(P)[:, None, None]
    m = np.arange(4)[None, :, None]
    c = np.arange(TQ_TILE)[None, None, :]
    masks = (c >= 128 * m + r - delta).astype(np.float32)
    ones = np.ones((P, P), dtype=np.float32)
    bias = np.ascontiguousarray(b_proj.reshape(KC, P).T)  # [P, KC]

    in_maps = []
    for core in range(NC):
        h0 = core * HL
        # q,k columns for my heads: [C, 4*P] -> [P, KC, 4P]
        qcols = W_qkv[:, h0 * D:(h0 + HL) * D]
        kcols = W_qkv[:, C + h0 * D:C + (h0 + HL) * D]
        wqk = np.ascontiguousarray(
            np.concatenate([qcols, kcols], axis=1)
            .reshape(KC, P, 4 * P).transpose(1, 0, 2))
        vcols = W_qkv[:, 2 * C + h0 * D:2 * C + (h0 + HL) * D]
        wv = np.ascontiguousarray(vcols.reshape(KC, P, HL * P).transpose(1, 0, 2))
        # cache k^T: [HL, B, P(D), Tc]
        kct = np.ascontiguousarray(
            cache_k[:, h0:h0 + HL].transpose(1, 0, 3, 2))
        vc = np.ascontiguousarray(cache_v[:, h0:h0 + HL].transpose(1, 0, 2, 3))
        in_maps.append({
            "xt": xt, "wqk": wqk, "wv": wv, "wp": wp,
            "kct": kct, "vc": vc, "masks": masks, "ones": ones, "bias": bias,
        })
    return in_maps


def _assemble(results, cache_k, cache_v):
    """Gather per-core outputs into full tensors."""
    out_T = np.concatenate([results[c]["out_t"] for c in range(NC)], axis=1)  # (C, B*T)
    out = np.ascontiguousarray(out_T.T).reshape(B, T, C)

    k_full = np.empty((B, H, Tc + T, D), dtype=np.float32)
    v_full = np.empty((B, H, Tc + T, D), dtype=np.float32)
    k_full[:, :, :Tc] = cache_k
    v_full[:, :, :Tc] = cache_v
    for c in range(NC):
        knt = results[c]["k_new_t"]  # [HL, B, D, T]
        vn = results[c]["v_new"]     # [HL, B, T, D]
        for h in range(HL):
            k_full[:, c * HL + h, Tc:] = knt[h].transpose(0, 2, 1)
            v_full[:, c * HL + h, Tc:] = vn[h]
    return out, k_full, v_full


def _run(inputs, trace=False, tmpdir=None, debug=False):
    seq_start = int(inputs["seq_start"])
    nc = _get_program(seq_start, debug)
    in_maps = _prep_inputs(**inputs)
    res = run_bass_kernel_spmd(
        nc, in_maps, list(range(NC)), trace=trace, tmpdir=tmpdir)
    out, k_full, v_full = _assemble(
        res.results,
        np.asarray(inputs["cache_k"], dtype=np.float32),
        np.asarray(inputs["cache_v"], dtype=np.float32))
    return (out, k_full, v_full), res


def kernel(**inputs):
    (out, k_full, v_full), _ = _run(inputs)
    return out, k_full, v_full


# revision 11
# speedup vs baseline: 1.1435x; 1.1435x over previous
"""Causal self-attention with KV cache on 8 Trainium2 NeuronCores.

Strategy: tensor-parallel over heads (16 heads / 8 cores = 2 heads per core).
  Per core:
    1. qkv^T projection: q^T,k^T per head (cols-on-partitions) and v (natural
       layout) via fp32r (TF32) matmuls against host-pre-transposed x^T.
    2. Flash-style attention in S^T layout (keys on partitions): S^T = k^T.T@q^T,
       exp on ACT (scale folded), causal mask via 0/1 mask multiply on diagonal
       blocks only, O^T = v.T@expS and row-sums l via ones-matmul, both
       accumulated in PSUM; normalize with reciprocal(l).
    3. Per-batch AllToAll (1 MB/core each) reshards O from head-parallel to
       sequence-parallel; batch-0's collective overlaps batch-1's attention,
       and batch-0's projection overlaps batch-1's collective.
    4. Output projection with the full W_proj: stationary = O^T chunks (reused
       across 4 moving W_proj column groups), output in natural [token, C]
       layout. Bias is added on the host.
  Host: shards/pre-transposes inputs, assembles outputs (+bias), passes the
  KV cache through unchanged.
"""
import numpy as np

import concourse.bass as bass
import concourse.tile as tile
from concourse import bacc, mybir
from concourse.bass_utils import run_bass_kernel_spmd

P = 128
NC = 8                     # cores
B, T, C = 2, 1024, 2048
H = 16
D = C // H                 # head dim 128
HL = H // NC               # heads per core = 2
Tc = 1024                  # cached tokens
KC = C // P                # contraction chunks = 16
TQ_TILE = 512
F32R = mybir.dt.float32r
F32 = mybir.dt.float32

_BUILD_CACHE = {}


def _build(seq_start: int):
    """Build the single SPMD program (all cores run it with different data)."""
    assert seq_start >= Tc - 1, "kernel assumes the whole cache is visible"
    delta = seq_start - Tc  # extra causal offset for new keys (0 in practice)
    n_new_blocks = T // P   # 8 new-key blocks per (head,batch)
    n_cache_blocks = Tc // P

    def visible_blocks(tq2: int):
        """tk blocks (of 16 = 8 cache + 8 new) visible to query tile tq2."""
        q_max = tq2 * TQ_TILE + TQ_TILE - 1
        blocks = list(range(n_cache_blocks))  # cache always fully visible
        for jn in range(n_new_blocks):
            if 128 * jn <= q_max + delta:
                blocks.append(n_cache_blocks + jn)
        return blocks

    def vis_col_start(j: int, tq2: int):
        """first query column of tile tq2 with any visible key in block j.
        Columns before it contribute nothing (fully masked) -> skip them."""
        if j < n_cache_blocks:
            return 0
        jn = j - n_cache_blocks
        c0 = 128 * jn - delta - tq2 * TQ_TILE  # col where key 128jn+0 unmasks
        return max(0, min(TQ_TILE - P, (c0 // P) * P))

    def is_diag(j: int, tq2: int):
        if j < n_cache_blocks:
            return False
        jn = j - n_cache_blocks
        return 128 * jn + 127 > tq2 * TQ_TILE + delta

    nc = bacc.Bacc("TRN2", target_bir_lowering=False, debug=False, num_devices=NC)

    # ---- DRAM I/O (per-core shards; same program on every core) ----
    xt_d = nc.dram_tensor("xt", [P, KC, B * T], F32R, kind="ExternalInput").ap()
    wqk_d = nc.dram_tensor("wqk", [P, KC, 4 * P], F32R, kind="ExternalInput").ap()
    wv_d = nc.dram_tensor("wv", [P, KC, HL * P], F32R, kind="ExternalInput").ap()
    wp_d = nc.dram_tensor("wp", [KC, P, C], F32R, kind="ExternalInput").ap()
    kct_d = nc.dram_tensor("kct", [HL, B, P, Tc], F32R, kind="ExternalInput").ap()
    vc_d = nc.dram_tensor("vc", [HL, B, Tc, D], F32R, kind="ExternalInput").ap()
    masks_d = nc.dram_tensor("masks", [P, 4, TQ_TILE], F32R, kind="ExternalInput").ap()
    ones_d = nc.dram_tensor("ones", [P, P], F32R, kind="ExternalInput").ap()

    out_d = nc.dram_tensor("out_t", [B, P, C], F32, kind="ExternalOutput").ap()
    kout_d = nc.dram_tensor("k_new_t", [HL, B, D, T], F32, kind="ExternalOutput").ap()
    vout_d = nc.dram_tensor("v_new", [HL, B, T, D], F32, kind="ExternalOutput").ap()

    # per-batch A2A buffers: shard s = (my 256 hd dims) x (core s's 128 tokens)
    a2a_in = [nc.dram_tensor(f"a2a_in{b}", [NC, HL * D, P], F32).ap() for b in range(B)]
    a2a_out = [nc.dram_tensor(f"a2a_out{b}", [NC, HL * D, P], F32).ap() for b in range(B)]

    with tile.TileContext(nc) as tc:
        with (
            tc.tile_pool(name="persist", bufs=1) as persist,
            tc.tile_pool(name="wp_pool", bufs=6) as wp_pool,
            tc.tile_pool(name="cache_pool", bufs=2) as cache_pool,
            tc.tile_pool(name="xt_pool", bufs=4) as xt_pool,
            tc.tile_pool(name="w1_pool", bufs=3) as w1_pool,
            tc.tile_pool(name="evict_pool", bufs=2) as evict_pool,
            tc.tile_pool(name="ps_big", bufs=1, space="PSUM") as ps_big,
            tc.tile_pool(name="ps_sm", bufs=2, space="PSUM") as ps_sm,
            tc.tile_pool(name="expS_pool", bufs=3) as expS_pool,
            tc.tile_pool(name="norm_pool", bufs=2) as norm_pool,
            tc.tile_pool(name="out_pool", bufs=2) as out_pool,
        ):
            # ---------- resident tiles ----------
            qT = persist.tile([P, HL, B, T], F32R)
            knew = persist.tile([P, HL, B, T], F32R)
            vnew = persist.tile([P, HL, B, T // P, D], F32R)
            masks_t = persist.tile([P, 4, TQ_TILE], F32R)
            ones_t = persist.tile([P, P], F32R)
            ofull = persist.tile([P, B, KC, P], F32R)   # post-A2A O^T per batch

            nc.sync.dma_start(masks_t[:], masks_d[:])
            nc.sync.dma_start(ones_t[:], ones_d[:])

            # ---------- phase 1: qkv^T projection ----------
            for tqi in range(B * T // TQ_TILE):
                b = tqi // 2
                q2 = tqi % 2
                psum_big = [ps_big.tile([P, 2 * TQ_TILE], F32, name=f"big{i}",
                                        tag=f"big{i}") for i in range(2)]
                psum_qk = [psum_big[m // 2][:, bass.ds((m % 2) * TQ_TILE, TQ_TILE)]
                           for m in range(4)]
                psum_v = [ps_sm.tile([P, HL * P], F32, name=f"v{i}", tag=f"sm{i % 2}")
                          for i in range(4)]
                for kc in range(KC):
                    xt_t = xt_pool.tile([P, TQ_TILE], F32R, tag="xt")
                    nc.sync.dma_start(xt_t[:], xt_d[:, kc, bass.ds(tqi * TQ_TILE, TQ_TILE)])
                    wqk_t = w1_pool.tile([P, 4 * P], F32R, tag="wqk")
                    nc.sync.dma_start(wqk_t[:], wqk_d[:, kc])
                    wv_t = w1_pool.tile([P, HL * P], F32R, tag="wv")
                    nc.sync.dma_start(wv_t[:], wv_d[:, kc])
                    for m in range(4):        # q0 q1 k0 k1
                        nc.tensor.matmul(
                            psum_qk[m][:], wqk_t[:, bass.ts(m, P)], xt_t[:],
                            start=(kc == 0), stop=(kc == KC - 1))
                    for tcc in range(4):
                        nc.tensor.matmul(
                            psum_v[tcc][:], xt_t[:, bass.ts(tcc, P)], wv_t[:],
                            start=(kc == 0), stop=(kc == KC - 1))
                # evictions: f32r working copies on ACT, fp32 output copies on DVE
                for h in range(HL):
                    nc.scalar.activation(
                        qT[:, h, b, bass.ds(q2 * TQ_TILE, TQ_TILE)], psum_qk[h][:],
                        mybir.ActivationFunctionType.Copy)
                    nc.scalar.activation(
                        knew[:, h, b, bass.ds(q2 * TQ_TILE, TQ_TILE)], psum_qk[2 + h][:],
                        mybir.ActivationFunctionType.Copy)
                    kout_t = evict_pool.tile([P, TQ_TILE], F32, tag="kout")
                    nc.vector.tensor_copy(kout_t[:], psum_qk[2 + h][:])
                    nc.sync.dma_start(
                        kout_d[h, b, :, bass.ds(q2 * TQ_TILE, TQ_TILE)], kout_t[:])
                for tcc in range(4):
                    blk = q2 * 4 + tcc
                    for h in range(HL):
                        nc.scalar.activation(
                            vnew[:, h, b, blk, :], psum_v[tcc][:, bass.ts(h, P)],
                            mybir.ActivationFunctionType.Copy)
                    vout_t = evict_pool.tile([P, HL * P], F32, tag="vout")
                    nc.vector.tensor_copy(vout_t[:], psum_v[tcc][:])
                    for h in range(HL):
                        nc.sync.dma_start(
                            vout_d[h, b, bass.ds(blk * P, P), :], vout_t[:, bass.ts(h, P)])

            # ---------- phase 2+3: attention, per-batch A2A, projection ----------
            scale = float(D) ** -0.5

            def attention(b):
                for h in range(HL):
                    kc_t = cache_pool.tile([P, Tc], F32R, tag="kcache")
                    nc.sync.dma_start(kc_t[:], kct_d[h, b])
                    vc_t = cache_pool.tile([P, Tc // P, D], F32R, tag="vcache")
                    nc.sync.dma_start(
                        vc_t[:], vc_d[h, b].rearrange("(blk p) d -> p blk d", p=P))

                    def k_blk(j):
                        if j < n_cache_blocks:
                            return kc_t[:, bass.ts(j, P)]
                        return knew[:, h, b, bass.ts(j - n_cache_blocks, P)]

                    def v_blk(j):
                        if j < n_cache_blocks:
                            return vc_t[:, j, :]
                        return vnew[:, h, b, j - n_cache_blocks, :]

                    for tq2 in range(T // TQ_TILE):
                        vis = visible_blocks(tq2)
                        pairs = [vis[i:i + 2] for i in range(0, len(vis), 2)]
                        psum_o = ps_sm.tile([P, TQ_TILE], F32, name="po", tag="sm0")
                        psum_l = ps_sm.tile([P, TQ_TILE], F32, name="pl", tag="sm1")
                        first, last = vis[0], vis[-1]
                        for pi, pr in enumerate(pairs):
                            psum_s = ps_big.tile([P, 2 * TQ_TILE], F32,
                                                 name="ps", tag=f"big{pi % 2}")
                            cs = [vis_col_start(j, tq2) for j in pr]
                            for e, j in enumerate(pr):
                                q_rhs = qT[:, h, b,
                                           bass.ds(tq2 * TQ_TILE + cs[e], TQ_TILE - cs[e])]
                                nc.tensor.matmul(
                                    psum_s[:, bass.ds(e * TQ_TILE + cs[e], TQ_TILE - cs[e])],
                                    k_blk(j), q_rhs, start=True, stop=True)
                            expS = expS_pool.tile([P, 2 * TQ_TILE], F32R, tag="expS")
                            cmin = min(cs)
                            nc.scalar.activation(
                                expS[:, bass.ds(cmin, 2 * TQ_TILE - cmin)],
                                psum_s[:, bass.ds(cmin, 2 * TQ_TILE - cmin)],
                                mybir.ActivationFunctionType.Exp, scale=scale)
                            for e, j in enumerate(pr):
                                if is_diag(j, tq2):
                                    m = (j - n_cache_blocks) - (tq2 * TQ_TILE) // P
                                    nc.vector.tensor_mul(
                                        out=expS[:, bass.ds(e * TQ_TILE + cs[e],
                                                            TQ_TILE - cs[e])],
                                        in0=expS[:, bass.ds(e * TQ_TILE + cs[e],
                                                            TQ_TILE - cs[e])],
                                        in1=masks_t[:, m, bass.ds(cs[e], TQ_TILE - cs[e])])
                            for e, j in enumerate(pr):
                                sub = expS[:, bass.ds(e * TQ_TILE + cs[e], TQ_TILE - cs[e])]
                                po = psum_o[:, bass.ds(cs[e], TQ_TILE - cs[e])]
                                pl = psum_l[:, bass.ds(cs[e], TQ_TILE - cs[e])]
                                nc.tensor.matmul(po, v_blk(j), sub,
                                                 start=(j == first), stop=(j == last))
                                nc.tensor.matmul(pl, ones_t[:], sub,
                                                 start=(j == first), stop=(j == last))
                        rl = norm_pool.tile([P, TQ_TILE], F32, tag="rl")
                        nc.vector.reciprocal(rl[:], psum_l[:])
                        oT = norm_pool.tile([P, TQ_TILE], F32R, tag="oT")
                        nc.vector.tensor_mul(out=oT[:], in0=psum_o[:], in1=rl[:])
                        s0 = tq2 * 4
                        nc.sync.dma_start(
                            a2a_in[b].bitcast(F32R)[
                                bass.ds(s0, 4), bass.ds(h * D, D), :]
                            .rearrange("s p t -> p s t"),
                            oT.rearrange("p (s t) -> p s t", s=4))

            def a2a(b):
                nc.gpsimd.collective_compute(
                    "AllToAll", mybir.AluOpType.bypass,
                    replica_groups=[list(range(NC))],
                    ins=[a2a_in[b][:]], outs=[a2a_out[b][:]])
                nc.sync.dma_start(
                    ofull[:, b],
                    a2a_out[b].bitcast(F32R).rearrange("s (o p) t -> p (s o) t", p=P))

            def proj():
                # out[b, tok, :] = sum_kc ofull[:, b, kc, :].T @ wp[kc].
                # Stationary = O^T chunk, reused over 4 moving 512-wide W_proj
                # column groups; one W_proj stream shared by both batches.
                # b=0 accumulators on the sm banks, b=1 on the big banks.
                psum_c0 = [ps_sm.tile([P, TQ_TILE], F32, name=f"pc0{cg}",
                                      tag=f"sm{cg % 2}") for cg in range(4)]
                psum_b1 = [ps_big.tile([P, 2 * TQ_TILE], F32, name=f"pcb{i}",
                                       tag=f"big{i}") for i in range(2)]
                psum_c1 = [psum_b1[cg // 2][:, bass.ds((cg % 2) * TQ_TILE, TQ_TILE)]
                           for cg in range(4)]
                psum_c = [psum_c0, psum_c1]
                for kc in range(KC):
                    wp_t = wp_pool.tile([P, C], F32R, tag="wp")
                    nc.sync.dma_start(wp_t[:], wp_d[kc])
                    for b in range(B):
                        for cg in range(4):
                            nc.tensor.matmul(
                                psum_c[b][cg][:], ofull[:, b, kc, :],
                                wp_t[:, bass.ts(cg, TQ_TILE)],
                                start=(kc == 0), stop=(kc == KC - 1))
                for b in range(B):
                    for cg in range(4):
                        outsb = out_pool.tile([P, TQ_TILE], F32, tag="osb")
                        nc.scalar.activation(outsb[:], psum_c[b][cg][:],
                                             mybir.ActivationFunctionType.Copy)
                        nc.sync.dma_start(
                            out_d[b, :, bass.ts(cg, TQ_TILE)], outsb[:])

            attention(0)
            a2a(0)
            attention(1)
            a2a(1)
            proj()

    nc.finalize()
    return nc


def _get_program(seq_start: int):
    if seq_start not in _BUILD_CACHE:
        _BUILD_CACHE[seq_start] = _build(seq_start)
    return _BUILD_CACHE[seq_start]


def _prep_inputs(x, cache_k, cache_v, W_qkv, W_proj, b_proj, seq_start):
    """Host-side sharding/layout prep. Returns in_maps for the 8 cores."""
    x = np.asarray(x, dtype=np.float32)
    cache_k = np.asarray(cache_k, dtype=np.float32)
    cache_v = np.asarray(cache_v, dtype=np.float32)
    W_qkv = np.asarray(W_qkv, dtype=np.float32)
    W_proj = np.asarray(W_proj, dtype=np.float32)
    delta = int(seq_start) - Tc

    xt = np.ascontiguousarray(
        x.reshape(B * T, C).T.reshape(KC, P, B * T).transpose(1, 0, 2))
    wp = np.ascontiguousarray(W_proj.reshape(KC, P, C))
    r = np.arange(P)[:, None, None]
    m = np.arange(4)[None, :, None]
    c = np.arange(TQ_TILE)[None, None, :]
    masks = (c >= 128 * m + r - delta).astype(np.float32)
    ones = np.ones((P, P), dtype=np.float32)

    in_maps = []
    for core in range(NC):
        h0 = core * HL
        qcols = W_qkv[:, h0 * D:(h0 + HL) * D]
        kcols = W_qkv[:, C + h0 * D:C + (h0 + HL) * D]
        wqk = np.ascontiguousarray(
            np.concatenate([qcols, kcols], axis=1)
            .reshape(KC, P, 4 * P).transpose(1, 0, 2))
        vcols = W_qkv[:, 2 * C + h0 * D:2 * C + (h0 + HL) * D]
        wv = np.ascontiguousarray(vcols.reshape(KC, P, HL * P).transpose(1, 0, 2))
        kct = np.ascontiguousarray(cache_k[:, h0:h0 + HL].transpose(1, 0, 3, 2))
        vc = np.ascontiguousarray(cache_v[:, h0:h0 + HL].transpose(1, 0, 2, 3))
        in_maps.append({
            "xt": xt, "wqk": wqk, "wv": wv, "wp": wp,
            "kct": kct, "vc": vc, "masks": masks, "ones": ones,
        })
    return in_maps


def _assemble(results, cache_k, cache_v, b_proj):
    """Gather per-core outputs into full tensors."""
    out = np.empty((B, T, C), dtype=np.float32)
    for c in range(NC):
        ot = results[c]["out_t"]          # [B, 128, C]
        out[:, c * P:(c + 1) * P, :] = ot
    out += np.asarray(b_proj, dtype=np.float32)[None, None, :]

    k_full = np.empty((B, H, Tc + T, D), dtype=np.float32)
    v_full = np.empty((B, H, Tc + T, D), dtype=np.float32)
    k_full[:, :, :Tc] = cache_k
    v_full[:, :, :Tc] = cache_v
    for c in range(NC):
        knt = results[c]["k_new_t"]  # [HL, B, D, T]
        vn = results[c]["v_new"]     # [HL, B, T, D]
        for h in range(HL):
            k_full[:, c * HL + h, Tc:] = knt[h].transpose(0, 2, 1)
            v_full[:, c * HL + h, Tc:] = vn[h]
    return out, k_full, v_full


def _run(inputs, trace=False, tmpdir=None):
    seq_start = int(inputs["seq_start"])
    nc = _get_program(seq_start)
    in_maps = _prep_inputs(**inputs)
    res = run_bass_kernel_spmd(
        nc, in_maps, list(range(NC)), trace=trace, tmpdir=tmpdir)
    out, k_full, v_full = _assemble(
        res.results,
        np.asarray(inputs["cache_k"], dtype=np.float32),
        np.asarray(inputs["cache_v"], dtype=np.float32),
        inputs["b_proj"])
    return (out, k_full, v_full), res


def kernel(**inputs):
    (out, k_full, v_full), _ = _run(inputs)
    return out, k_full, v_full
